# revision 2
# baseline (speedup 1.0000x reference)
"""Llama GQA attention layer (B=1, S=2048, D=4096, H=32, KVH=8, DH=128) on 8 trn2 cores.

Sharding: tensor-parallel over heads. Core c owns Q heads [4c, 4c+4) and KV head c:
  Wq[:, c*512:(c+1)*512], Wk/Wv[:, c*128:(c+1)*128], Wo columns [c*512:(c+1)*512].

Data logistics are the dominant cost on this axon-tunneled setup (~55 MB/s to the
device), so the kernel minimizes host<->device bytes:
  - hidden_states / cos / sin / mask are SEQUENCE-SHARDED on the host (each core
    receives one packed [128, 8960] bf16 block = its 256-seq slice) and
    AllGathered on device (62 GB/s ring) into the full X^T layout.
  - Wo is sharded COLUMN-wise; after attention, the per-head attention outputs
    (at, bf16) are AllGathered on device so each core computes a DISJOINT
    [2048, 512] output slice. No host-side reduction, no fp32 partials: the
    output download is 8 x 2 MB bf16 instead of 8 x 32 MB fp32 (and the donated
    zero output buffers shrink identically).

Kernel layout strategy (per core):
  - X^T [4096, 2048] from the gathered block; projections computed as
    Q^T/K^T/V^T [dh, s] via PSUM accumulation over 32 d-tiles.
  - RoPE applied on PSUM evacuation (DVE, partition-half shuffle).
  - V^T transposed to V natural [s, dh] via PE-transpose (needed as PV stationary).
  - Attention with scores TRANSPOSED: S^T[k, q] tiles [128, 512] so softmax sums
    over keys become ones-vector matmuls; exp on ACT (no max subtraction - scores
    are O(10), exp is safe); causal sparsity by skipping fully-masked key tiles;
    diagonal tiles masked multiplicatively with 4 static 0/1 tiles.
  - Softmax normalization: recip of sums row [1,512] broadcast across partitions
    via a K=1 ones matmul, then one DVE mul per attn^T tile.
  - at AllGather, then output projection over all 32 heads into this core's
    512 output columns, streamed out as bf16.
"""

import os as _os

# Persistent XLA compilation cache: the per-call jax.jit inside
# run_bass_kernel_spmd re-lowers the same module every call; cache it.
_os.environ.setdefault("JAX_COMPILATION_CACHE_DIR", "/tmp/jax_bass_cache")

import numpy as np

import concourse.bass as bass
import concourse.bacc as bacc
import concourse.mybir as mybir
import concourse.tile as tile
from concourse.bass_utils import run_bass_kernel_spmd

try:
    import jax as _jax
    _jax.config.update("jax_persistent_cache_min_entry_size_bytes", 0)
    _jax.config.update("jax_persistent_cache_min_compile_time_secs", 0.0)
except Exception:
    pass

S = 2048
D = 4096
H = 32
KVH = 8
DH = 128
NCORES = 8
HPC = H // NCORES            # 4 query heads per core
QC = HPC * DH                # 512 projection cols per core
SCALE = float(DH) ** -0.5
NT_D = D // 128              # 32 contraction tiles
NCH = S // 512               # 4 sequence chunks
SPC = S // NCORES            # 256 seq positions shipped per core
FP32 = mybir.dt.float32
FP32R = mybir.dt.float32r
BF16 = mybir.dt.bfloat16
AF = mybir.ActivationFunctionType

MMDT = {"bf16": BF16, "fp32r": FP32R}[_os.environ.get("KERNEL_MM_DTYPE", "bf16")]

# packed per-core input block: [xt | cos | sin | mask] along free dim
XT_W = NT_D * SPC            # 8192
CS_O = XT_W                  # cos at [8192, 8448)
SN_O = XT_W + SPC            # sin at [8448, 8704)
MK_O = XT_W + 2 * SPC        # mask at [8704, 8960)
PACK_W = XT_W + 3 * SPC      # 8960


def _np_mmdt():
    import ml_dtypes
    return {BF16: ml_dtypes.bfloat16, FP32R: np.float32}[MMDT]


def _emit(nc, tc, io, mode, phases="ABC"):
    """mode: 'causal' (sparse, static diag masks), 'dense' (all tiles, no mask),
    'masked' (all tiles, additive mask streamed from DRAM)."""
    from contextlib import ExitStack

    xcs_d, wq_d, wk_d, wv_d, wo_d, msk_d, id_d, on_d, out_d = io

    with ExitStack() as top:
        ep = top.enter_context  # persistent pools

        # ---------- persistent SBUF (whole kernel) ----------
        pers = ep(tc.tile_pool(name="pers", bufs=1))
        qt = pers.tile([128, HPC * S], MMDT, name="qt")        # Q^T, head h at [:, h*S:(h+1)*S]
        kt = pers.tile([128, S], MMDT, name="kt")              # K^T
        vn = pers.tile([128, S], MMDT, name="vn")              # V natural, tile t at [:, 128t:128t+128]
        at = pers.tile([128, HPC * S], MMDT, name="at")        # attn^T
        ones_c = pers.tile([128, 1], MMDT, name="ones_c")
        ones_r = pers.tile([1, 128], FP32, name="ones_r")
        msk_sb = pers.tile([128, 4 * 512], MMDT, name="msk_sb")

        dram = ep(tc.tile_pool(name="dram", bufs=1, space="DRAM"))
        gx = dram.tile([NCORES * 128, PACK_W], MMDT, name="gx")       # gathered packed input
        gx_in = dram.tile([128, PACK_W], MMDT, name="gx_in")
        gat = dram.tile([NCORES * 128, HPC * S], MMDT, name="gat")    # gathered attn^T
        gat_in = dram.tile([128, HPC * S], MMDT, name="gat_in")

        # input AllGather first thing; weight DMAs below overlap with it
        nc.sync.dma_start(gx_in[:], xcs_d[:])
        nc.gpsimd.collective_compute(
            "AllGather", mybir.AluOpType.bypass,
            replica_groups=[list(range(NCORES))],
            ins=[gx_in.opt()], outs=[gx.opt()])

        # ================= Phase A: projections =================
        with ExitStack() as pa:
            e = pa.enter_context
            wpool = e(tc.tile_pool(name="wpool", bufs=1))
            id_sb = wpool.tile([128, 128], MMDT, name="id_sb")
            nc.sync.dma_start(id_sb[:], id_d[:])
            csb = wpool.tile([128, S], MMDT, name="csb")
            snb = wpool.tile([128, S], MMDT, name="snb")
            cs_sb = wpool.tile([128, S], FP32, name="cs_sb")
            sn_sb = wpool.tile([128, S], FP32, name="sn_sb")
            xpool = e(tc.tile_pool(name="xpool", bufs=3))
            tpool = e(tc.tile_pool(name="tpool", bufs=2))
            psum = e(tc.tile_pool(name="psumA", bufs=1, space=bass.MemorySpace.PSUM))

            wq_t2 = [wpool.tile([128, 2 * QC], MMDT, name=f"wq2_{i}")
                     for i in range(NT_D // 2)]
            wk_t8 = [wpool.tile([128, 8 * DH], MMDT, name=f"wk8_{i}")
                     for i in range(NT_D // 8)]
            wv_t8 = [wpool.tile([128, 8 * DH], MMDT, name=f"wv8_{i}")
                     for i in range(NT_D // 8)]
            for i in range(NT_D // 2):
                nc.sync.dma_start(wq_t2[i][:], wq_d[:, i * 2 * QC:(i + 1) * 2 * QC])
            for i in range(NT_D // 8):
                nc.sync.dma_start(wk_t8[i][:], wk_d[:, i * 8 * DH:(i + 1) * 8 * DH])
                nc.sync.dma_start(wv_t8[i][:], wv_d[:, i * 8 * DH:(i + 1) * 8 * DH])
            nc.sync.dma_start(ones_c[:], on_d[:])
            nc.vector.memset(ones_r[:], 1.0)
            # unpack cos/sin/mask from the gathered blocks
            for b in range(NCORES):
                rr = slice(b * 128, (b + 1) * 128)
                cc = slice(b * SPC, (b + 1) * SPC)
                nc.sync.dma_start(csb[:, cc], gx[rr, CS_O:CS_O + SPC])
                nc.sync.dma_start(snb[:, cc], gx[rr, SN_O:SN_O + SPC])
                if mode == "causal":
                    nc.sync.dma_start(msk_sb[:, cc], gx[rr, MK_O:MK_O + SPC])
            nc.vector.tensor_copy(cs_sb[:], csb[:])
            nc.vector.tensor_copy(sn_sb[:], snb[:])

            def wq_ap(dt_, h):
                return wq_t2[dt_ // 2][:, (dt_ % 2) * QC + h * 128:
                                       (dt_ % 2) * QC + (h + 1) * 128]

            def wk_ap(dt_):
                return wk_t8[dt_ // 8][:, (dt_ % 8) * DH:(dt_ % 8 + 1) * DH]

            def wv_ap(dt_):
                return wv_t8[dt_ // 8][:, (dt_ % 8) * DH:(dt_ % 8 + 1) * DH]

            def rope_evac(src_ps, dest, ci):
                cs = cs_sb[:, ci * 512:(ci + 1) * 512]
                sn = sn_sb[:, ci * 512:(ci + 1) * 512]
                t1 = tpool.tile([128, 512], FP32, tag="t1", bufs=2)
                t2 = tpool.tile([128, 512], FP32, tag="t2", bufs=2)
                nc.vector.tensor_mul(t1[:], src_ps[:], cs)
                nc.vector.tensor_mul(t2[0:64, :], src_ps[64:128, :], sn[0:64, :])
                nc.vector.tensor_mul(t2[64:128, :], src_ps[0:64, :], sn[64:128, :])
                nc.vector.tensor_sub(dest[0:64, :], t1[0:64, :], t2[0:64, :])
                nc.vector.tensor_add(dest[64:128, :], t1[64:128, :], t2[64:128, :])

            for ci in range(NCH):
                acc = [psum.tile([128, 512], FP32, tag="acc", bufs=6,
                                 name=f"acc{ci}_{b}") for b in range(6)]
                for i in range(NT_D // 2):
                    xt_t = xpool.tile([128, 1024], MMDT, tag="xt", bufs=4)
                    # [128, 1024] = (dt=2i, dt=2i+1) x (two 256-wide rank blocks)
                    for half in range(2):
                        dt_ = 2 * i + half
                        for k in range(2):
                            b = 2 * ci + k
                            nc.sync.dma_start(
                                xt_t[:, half * 512 + k * SPC:
                                     half * 512 + (k + 1) * SPC],
                                gx[b * 128:(b + 1) * 128,
                                   dt_ * SPC:(dt_ + 1) * SPC])
                    for half in range(2):
                        dt_ = 2 * i + half
                        st = dt_ == 0
                        sp = dt_ == NT_D - 1
                        rhs = xt_t[:, half * 512:(half + 1) * 512]
                        for h in range(HPC):
                            nc.tensor.matmul(acc[h][:], wq_ap(dt_, h), rhs,
                                             start=st, stop=sp)
                        nc.tensor.matmul(acc[4][:], wk_ap(dt_), rhs,
                                         start=st, stop=sp)
                        nc.tensor.matmul(acc[5][:], wv_ap(dt_), rhs,
                                         start=st, stop=sp)
                for h in range(HPC):
                    rope_evac(acc[h], qt[:, h * S + ci * 512:h * S + (ci + 1) * 512], ci)
                rope_evac(acc[4], kt[:, ci * 512:(ci + 1) * 512], ci)
                # V: plain evac then PE-transpose each 128 block to natural layout
                vt_t = tpool.tile([128, 512], MMDT, tag="vt", bufs=2)
                nc.scalar.copy(vt_t[:], acc[5][:])
                for i in range(4):
                    ps_tr = psum.tile([128, 128], MMDT, tag="tr", bufs=2,
                                      name=f"tr{ci}_{i}")
                    nc.tensor.transpose(ps_tr[:], vt_t[:, i * 128:(i + 1) * 128], id_sb[:])
                    s0 = (ci * 4 + i) * 128
                    nc.vector.tensor_copy(vn[:, s0:s0 + 128], ps_tr[:])

        if "B" not in phases:
            return

        # ================= Phase B: attention =================
        with ExitStack() as pb:
            e = pb.enter_context
            ppool = e(tc.tile_pool(name="ppool", bufs=4))
            npool = e(tc.tile_pool(name="npool", bufs=2))
            mpool = e(tc.tile_pool(name="mpool", bufs=4))
            psum = e(tc.tile_pool(name="psumB", bufs=1, space=bass.MemorySpace.PSUM))

            for ci in range(NCH):
                n_sk = 4 * (ci + 1) if mode == "causal" else S // 128
                for h in range(HPC):
                    ps_pv = psum.tile([128, 512], FP32, tag="pv", bufs=2,
                                      name=f"pv{ci}_{h}")
                    ps_sm = psum.tile([1, 512], FP32, tag="sm", bufs=2,
                                      name=f"sm{ci}_{h}")
                    qs = qt[:, h * S + ci * 512:h * S + (ci + 1) * 512]
                    for sk in range(n_sk):
                        ps_sc = psum.tile([128, 512], FP32, tag="sc", bufs=2,
                                          name=f"sc{ci}_{h}_{sk}")
                        nc.tensor.matmul(ps_sc[:], kt[:, sk * 128:(sk + 1) * 128],
                                         qs, start=True, stop=True)
                        p = ppool.tile([128, 512], MMDT, tag="p", bufs=4)
                        if mode == "masked":
                            mt = mpool.tile([128, 512], FP32, tag="mt", bufs=4)
                            nc.sync.dma_start(
                                mt[:], msk_d[sk * 128:(sk + 1) * 128,
                                             ci * 512:(ci + 1) * 512])
                            nc.vector.tensor_scalar_mul(p[:], ps_sc[:], SCALE)
                            nc.vector.tensor_add(p[:], p[:], mt[:])
                            nc.scalar.activation(p[:], p[:], AF.Exp)
                        else:
                            nc.scalar.activation(p[:], ps_sc[:], AF.Exp, scale=SCALE)
                            if mode == "causal" and sk >= 4 * ci:
                                j = sk - 4 * ci
                                nc.vector.tensor_mul(
                                    p[:], p[:], msk_sb[:, j * 512:(j + 1) * 512])
                        st = sk == 0
                        sp = sk == n_sk - 1
                        nc.tensor.matmul(ps_pv[:], vn[:, sk * 128:(sk + 1) * 128],
                                         p[:], start=st, stop=sp)
                        nc.tensor.matmul(ps_sm[:], ones_c[:], p[:],
                                         start=st, stop=sp)
                    # normalize: 1/sums broadcast over partitions via K=1 matmul
                    rc = npool.tile([1, 512], FP32, tag="rc", bufs=2)
                    rs = npool.tile([1, 512], FP32, tag="rs", bufs=2)
                    nc.vector.reciprocal_approx_accurate(rc[:], ps_sm[:], rs[:])
                    ps_bc = psum.tile([128, 512], FP32, tag="bc", bufs=2,
                                      name=f"bc{ci}_{h}")
                    nc.tensor.matmul(ps_bc[:], ones_r[:], rc[:], start=True, stop=True)
                    rb = npool.tile([128, 512], FP32, tag="rb", bufs=2)
                    nc.scalar.copy(rb[:], ps_bc[:])
                    nc.vector.tensor_mul(at[:, h * S + ci * 512:h * S + (ci + 1) * 512],
                                         ps_pv[:], rb[:])

        if "C" not in phases:
            return
        # ================= Phase C: gather heads, project this core's columns ====
        nc.sync.dma_start(gat_in[:], at[:])
        nc.gpsimd.collective_compute(
            "AllGather", mybir.AluOpType.bypass,
            replica_groups=[list(range(NCORES))],
            ins=[gat_in.opt()], outs=[gat.opt()])
        with ExitStack() as pc:
            e = pc.enter_context
            wopool = e(tc.tile_pool(name="wopool", bufs=1))
            apool = e(tc.tile_pool(name="apool", bufs=2))
            opool = e(tc.tile_pool(name="opool", bufs=4))
            psum = e(tc.tile_pool(name="psumC", bufs=1, space=bass.MemorySpace.PSUM))
            wo_sb = wopool.tile([128, H * 512], MMDT, name="wo_sb")
            for i in range(4):
                nc.sync.dma_start(wo_sb[:, i * 8 * 512:(i + 1) * 8 * 512],
                                  wo_d[:, i * 8 * 512:(i + 1) * 8 * 512])
            for sb in range(S // 128):
                at_sb = apool.tile([128, H * 128], MMDT, tag="at_sb", bufs=2)
                for hh in range(H):
                    b, sub = hh // HPC, hh % HPC
                    nc.sync.dma_start(
                        at_sb[:, hh * 128:(hh + 1) * 128],
                        gat[b * 128:(b + 1) * 128,
                            sub * S + sb * 128:sub * S + (sb + 1) * 128])
                ps_o = psum.tile([128, 512], FP32, tag="oo", bufs=4,
                                 name=f"oo{sb}")
                for hh in range(H):
                    nc.tensor.matmul(ps_o[:],
                                     at_sb[:, hh * 128:(hh + 1) * 128],
                                     wo_sb[:, hh * 512:(hh + 1) * 512],
                                     start=(hh == 0), stop=(hh == H - 1))
                ob = opool.tile([128, 512], MMDT, tag="ob", bufs=4)
                nc.vector.tensor_copy(ob[:], ps_o[:])
                nc.sync.dma_start(out_d[sb * 128:(sb + 1) * 128, :], ob[:])


def build(mode="causal", phases="ABC"):
    nc = bacc.Bacc("TRN2", target_bir_lowering=False, debug=False,
                   num_devices=NCORES)
    xcs_d = nc.dram_tensor("xcs", [128, PACK_W], MMDT, kind="ExternalInput").ap()
    wq_d = nc.dram_tensor("wq", [128, NT_D * QC], MMDT, kind="ExternalInput").ap()
    wk_d = nc.dram_tensor("wk", [128, NT_D * DH], MMDT, kind="ExternalInput").ap()
    wv_d = nc.dram_tensor("wv", [128, NT_D * DH], MMDT, kind="ExternalInput").ap()
    wo_d = nc.dram_tensor("wo", [128, H * 512], MMDT, kind="ExternalInput").ap()
    # masked: [S, S] additive mask^T streamed from DRAM (otherwise unused dummy)
    mshape2 = [S, S] if mode == "masked" else [1, 1]
    msk_d = nc.dram_tensor("msk", mshape2, FP32, kind="ExternalInput").ap()
    id_d = nc.dram_tensor("ident", [128, 128], MMDT, kind="ExternalInput").ap()
    on_d = nc.dram_tensor("ones", [128, 1], MMDT, kind="ExternalInput").ap()
    out_d = nc.dram_tensor("out", [S, QC], MMDT, kind="ExternalOutput").ap()
    io = (xcs_d, wq_d, wk_d, wv_d, wo_d, msk_d, id_d, on_d, out_d)
    with tile.TileContext(nc) as tc:
        _emit(nc, tc, io, mode, phases)
    nc.compile()
    return nc


_CACHE = {}
RUN_KWARGS = {}   # extra kwargs for run_bass_kernel_spmd (e.g. trace=True)
LAST = None       # last BassKernelResults (for exec_time_ns inspection)


def _causal_ref_mask():
    neg = np.finfo(np.float32).min
    m = np.where(np.tril(np.ones((S, S), dtype=bool)), 0.0, neg)
    return m.astype(np.float32)


def _tile_rows(w):
    # [T*128, C] -> [128, T*C] with d-tile blocks along free dim
    t = w.shape[0] // 128
    return np.ascontiguousarray(
        w.reshape(t, 128, w.shape[1]).transpose(1, 0, 2).reshape(128, -1))


def make_in_maps(hidden_states, cos, sin, attention_mask, Wq, Wk, Wv, Wo, mode):
    mdt = _np_mmdt()
    xtb = np.asarray(hidden_states).reshape(S, D).T.astype(mdt)   # [4096, 2048]
    xblk = xtb.reshape(NT_D, 128, S)                              # [32, 128, 2048]
    cosT = np.asarray(cos).T.astype(mdt)                          # [128, 2048]
    sinT = np.asarray(sin).T.astype(mdt)
    ident = np.eye(128, dtype=mdt)
    if mode == "masked":
        msk = np.ascontiguousarray(
            np.asarray(attention_mask).reshape(S, S).T).astype(np.float32)
    else:
        msk = np.zeros((1, 1), dtype=np.float32)
    if mode == "causal":
        # 4 diagonal 0/1 tiles: tile j valid where 128*j + k <= q  (k:[128], q:[512])
        j = np.arange(4)[:, None, None]
        k = np.arange(128)[None, :, None]
        q = np.arange(512)[None, None, :]
        mflat = np.ascontiguousarray((128 * j + k <= q).astype(mdt)
                                     .transpose(1, 0, 2).reshape(128, 4 * 512))
    else:
        mflat = np.zeros((128, 4 * 512), dtype=mdt)
    ones = np.ones((128, 1), dtype=mdt)
    in_maps = []
    for c in range(NCORES):
        cc = slice(c * SPC, (c + 1) * SPC)
        packed = np.empty((128, PACK_W), dtype=mdt)
        packed[:, :XT_W] = xblk[:, :, cc].transpose(1, 0, 2).reshape(128, XT_W)
        packed[:, CS_O:CS_O + SPC] = cosT[:, cc]
        packed[:, SN_O:SN_O + SPC] = sinT[:, cc]
        packed[:, MK_O:MK_O + SPC] = mflat[:, cc]
        in_maps.append({
            "xcs": packed,
            "wq": _tile_rows(np.asarray(Wq[:, c * QC:(c + 1) * QC]).astype(mdt)),
            "wk": _tile_rows(np.asarray(Wk[:, c * DH:(c + 1) * DH]).astype(mdt)),
            "wv": _tile_rows(np.asarray(Wv[:, c * DH:(c + 1) * DH]).astype(mdt)),
            "wo": _tile_rows(np.asarray(Wo[:, c * QC:(c + 1) * QC]).astype(mdt)),
            "msk": msk, "ident": ident, "ones": ones,
        })
    return in_maps


def pick_mode(attention_mask):
    am = np.asarray(attention_mask).reshape(S, S)
    if np.array_equal(am, _causal_ref_mask()):
        return "causal"
    if not np.any(am):
        return "dense"
    return "masked"


def kernel(hidden_states, cos, sin, attention_mask, Wq, Wk, Wv, Wo, **kwargs):
    mode = pick_mode(attention_mask)
    ck = (mode, str(MMDT))
    if ck not in _CACHE:
        _CACHE[ck] = build(mode)
    nc = _CACHE[ck]
    in_maps = make_in_maps(hidden_states, cos, sin, attention_mask,
                           Wq, Wk, Wv, Wo, mode)
    res = run_bass_kernel_spmd(nc, in_maps, core_ids=list(range(NCORES)),
                               **RUN_KWARGS)
    global LAST
    LAST = res
    out = np.concatenate([res.results[c]["out"] for c in range(NCORES)], axis=1)
    return out.astype(np.float32).reshape(1, S, D)
